# revision 20
# baseline (speedup 1.0000x reference)
"""Trainium2 Bass kernel for the ESN (leaky-tanh echo state network) scan.

Problem: y[t] = mask_t * h_t ;  h_t = (1-L)h + L*tanh(x_t@W_ih.T + h@W_hh.T), L=0.5
Shapes: x [1000, 64, 80], lengths [64] (descending), W_ih [1024, 80], W_hh [1024, 1024].

Strategy (data-parallel over batch, 8 rows/core on 8 cores):
  The scan is PE-ingest bound on W_hh (1M elements/step). W_hh.T streams as the
  MOVING operand in float32r across 4 concurrent column-tiled matmul groups
  (tile_position=(0,32j)), each producing a 256-wide h' chunk for all batch rows.
  State g = 2h is kept in a "quad" layout g_q[32j+8r+b, n] = g[b, 256j+n]
  (4 replicas r so all 128 partitions hold real data), and its 32x32
  block-transpose gQT feeds the matmuls as the stationary operand; the W rows are
  pre-permuted on the host to match the block-transposed index order.
  i2h = x@W_ih.T is folded into the PSUM accumulation via small identity matmuls;
  masking is folded into tanh's per-partition scale (tanh(m*pre)) and the leak
  into the weights (g = 2h absorbs the outer 0.5).
  i2h is computed on-device (phase A) one window ahead into DRAM-pool tiles.
"""
import sys
import os

sys.path.insert(0, '/opt/trn_rl_repo')

import numpy as np
import concourse.bass as bass
import concourse.bacc as bacc
import concourse.mybir as mybir
import concourse.tile as tile
from contextlib import ExitStack

F32 = mybir.dt.float32
F32R = mybir.dt.float32r
BF16 = mybir.dt.bfloat16
AF = mybir.ActivationFunctionType

T, B, I_IN, H = 1000, 64, 80, 1024
NCORES = 8
BS = B // NCORES      # batch rows per core
LEAK = 0.5
WIN = 64              # i2h pipeline window (time steps)


def build_program(t_steps=T, win=WIN):
    nc = bacc.Bacc(None, target_bir_lowering=False)

    x_d = nc.dram_tensor("x", [t_steps * BS, I_IN], F32R, kind="ExternalInput")
    wq_d = nc.dram_tensor("wq", [128, 8 * H], BF16, kind="ExternalInput")
    wihT_d = nc.dram_tensor("wihT", [I_IN, H], F32R, kind="ExternalInput")
    mt_d = nc.dram_tensor("m_tbl", [128, t_steps], F32, kind="ExternalInput")
    at_d = nc.dram_tensor("a_tbl", [128, t_steps], F32, kind="ExternalInput")
    ht_d = nc.dram_tensor("mh_tbl", [128, t_steps], F32, kind="ExternalInput")
    eye_d = nc.dram_tensor("eye8x32", [BS, 32], BF16, kind="ExternalInput")
    id128_d = nc.dram_tensor("id128", [128, 128], F32R, kind="ExternalInput")
    y_d = nc.dram_tensor("y", [t_steps, BS, H], F32, kind="ExternalOutput")

    n_win = (t_steps + win - 1) // win

    with tile.TileContext(nc) as tc, ExitStack() as ctx:
        consts = ctx.enter_context(tc.tile_pool(name="consts", bufs=1))
        state = ctx.enter_context(tc.tile_pool(name="state", bufs=2))
        work = ctx.enter_context(tc.tile_pool(name="work", bufs=2))
        i2hsb = ctx.enter_context(tc.tile_pool(name="i2hsb", bufs=3))
        xload = ctx.enter_context(tc.tile_pool(name="xload", bufs=3))
        prep = ctx.enter_context(tc.tile_pool(name="prep", bufs=2, space="PSUM"))
        apsum = ctx.enter_context(tc.tile_pool(name="apsum", bufs=1, space="PSUM"))
        tpsum = ctx.enter_context(tc.tile_pool(name="tpsum", bufs=2, space="PSUM"))
        i2hdram = ctx.enter_context(tc.tile_pool(name="i2hdram", bufs=3, space="DRAM"))

        # ---- constants ----
        wq_sb = consts.tile([128, 8 * H], BF16)
        nc.sync.dma_start(out=wq_sb, in_=wq_d[:, :])
        wihT_sb = consts.tile([I_IN, H], F32R)
        nc.sync.dma_start(out=wihT_sb, in_=wihT_d[:, :])
        m_sb = consts.tile([128, t_steps], F32)
        nc.sync.dma_start(out=m_sb, in_=mt_d[:, :])
        a_sb = consts.tile([128, t_steps], F32)
        nc.sync.dma_start(out=a_sb, in_=at_d[:, :])
        mh_sb = consts.tile([128, t_steps], F32)
        nc.sync.dma_start(out=mh_sb, in_=ht_d[:, :])
        eye_sb = consts.tile([BS, 32], BF16)
        nc.sync.dma_start(out=eye_sb, in_=eye_d[:, :])
        id_sb = consts.tile([128, 128], F32R)
        nc.sync.dma_start(out=id_sb, in_=id128_d[:, :])
        zl_sb = consts.tile([1, 128], BF16)
        nc.vector.memset(zl_sb, 0.0)
        zr_sb = consts.tile([1, 256], BF16)
        nc.vector.memset(zr_sb, 0.0)

        # ---- initial state ----
        g_cur = state.tile([128, 256], BF16, tag="g")
        nc.vector.memset(g_cur, 0.0)
        gqt_cur = state.tile([128, 256], BF16, tag="gqt")
        nc.vector.memset(gqt_cur, 0.0)
        # one-time DVE reads of the step tables so their DMA waits don't land
        # on per-step TensorScalarPtr ops (which allow a single sync wait).
        tbl_warm = consts.tile([128, 4], F32)
        nc.vector.tensor_copy(tbl_warm[:, 0:1], m_sb[:, 0:1])
        nc.vector.tensor_copy(tbl_warm[:, 1:2], a_sb[:, 0:1])
        nc.vector.tensor_copy(tbl_warm[:, 2:3], mh_sb[:, 0:1])

        win_tiles = {}

        def phase_a(w):
            t0 = w * win
            nt = min(win, t_steps - t0)
            rows = nt * BS
            dtile = i2hdram.tile([win * BS, H], BF16, tag="i2hwin")
            win_tiles[w] = dtile
            r = 0
            while r < rows:
                cr = min(128, rows - r)
                x_sb = xload.tile([128, I_IN], F32R, tag="xc")
                nc.sync.dma_start(out=x_sb[:cr, :],
                                  in_=x_d[t0 * BS + r: t0 * BS + r + cr, :])
                xt_ps = tpsum.tile([128, 128], F32R, tag="xtp")
                # bf16 zero-matmul claims the psum bank and absorbs the
                # cross-engine waits (x DMA + psum WAR) that the fp32-family
                # transpose below could not carry (S3_LW allows one wait).
                nc.tensor.matmul(xt_ps.bitcast(F32)[:, :], zl_sb[:, :],
                                 id_sb.bitcast(BF16)[0:1, 1:256:2],
                                 start=True, stop=False, tile_position=(0, 0))
                nc.tensor.matmul(xt_ps[:I_IN, :cr], x_sb[:cr, :I_IN],
                                 id_sb[:cr, :cr], is_transpose=True,
                                 start=False, stop=False)
                nc.tensor.matmul(xt_ps.bitcast(F32)[:, 0:1], zl_sb[:, :],
                                 zr_sb[0:1, 0:1], start=False, stop=True,
                                 tile_position=(0, 0))
                xt_sb = xload.tile([I_IN, 128], F32R, tag="xt")
                nc.scalar.copy(out=xt_sb[:, :cr], in_=xt_ps[:I_IN, :cr])
                for half in range(2):
                    psh = apsum.tile([128, 512], F32, tag=f"aps{half}")
                    nc.tensor.matmul(
                        psh[:, :], zl_sb[:, :],
                        wihT_sb.bitcast(BF16)[0:1, half * 1024 + 1:
                                              (half + 1) * 1024:2],
                        start=True, stop=False, tile_position=(0, 0))
                    nc.tensor.matmul(
                        psh[:cr, :],
                        xt_sb[:, :cr],
                        wihT_sb[:, half * 512:(half + 1) * 512],
                        start=False, stop=False,
                    )
                    nc.tensor.matmul(psh[:, 0:1], zl_sb[:, :],
                                     zr_sb[0:1, 0:1], start=False, stop=True,
                                     tile_position=(0, 0))
                    cp = xload.tile([128, 512], BF16, tag=f"i2hcp{half}")
                    if half == 0:
                        nc.scalar.copy(out=cp[:cr, :], in_=psh[:cr, :])
                    else:
                        nc.vector.tensor_copy(cp[:cr, :], psh[:cr, :])
                    nc.sync.dma_start(
                        out=dtile[r:r + cr, half * 512:(half + 1) * 512],
                        in_=cp[:cr, :])
                r += cr

        def scan_step(t):
            nonlocal g_cur, gqt_cur
            w, tl = divmod(t, win)
            dtile = win_tiles[w]
            i2h_sb = i2hsb.tile([BS, H], BF16, tag="i2h")
            nc.sync.dma_start(out=i2h_sb, in_=dtile[tl * BS:(tl + 1) * BS, :])

            pre = prep.tile([128, 256], F32, tag="pre")
            # bf16 zero-matmuls: claim the psum bank with a single start=True
            # (the four col-group chains below all use start=False and rely on
            # per-element has_written overwrite/accumulate), and absorb the
            # cross-engine waits (DVE transposes, i2h DMA, psum WAR) that the
            # 4-byte-weight f32r matmuls cannot carry (S3_LW: one wait).
            nc.tensor.matmul(pre[:, :], zl_sb[:, :],
                             zr_sb[0:1, 0:256],
                             start=True, stop=False, tile_position=(0, 0))
            nc.tensor.matmul(pre[:, 0:1], zl_sb[:, :],
                             i2h_sb[0:1, 0:1],
                             start=False, stop=False, tile_position=(0, 0))
            if t == 0:
                nc.tensor.matmul(pre[:, :], zl_sb[:, :],
                                 wq_sb[0:1, 0:256],
                                 start=False, stop=False, tile_position=(0, 0))
            for g in range(8):
                for j in range(4):
                    nc.tensor.matmul(
                        pre[32 * j:32 * j + 32, :],
                        gqt_cur[:, 32 * g:32 * g + 32],
                        wq_sb[:, g * H + j * 256:g * H + j * 256 + 256],
                        start=False, stop=False,
                        tile_position=(0, 32 * j),
                    )
            for j in range(4):
                nc.tensor.matmul(
                    pre[32 * j:32 * j + 32, :],
                    eye_sb[:, :],
                    i2h_sb[:, j * 256:(j + 1) * 256],
                    start=False, stop=False,
                    tile_position=(0, 32 * j),
                )
            nc.tensor.matmul(pre[:, 0:1], zl_sb[:, :], zr_sb[0:1, 0:1],
                             start=False, stop=True, tile_position=(0, 0))

            th = work.tile([128, 256], F32, tag="th")
            for s in range(4):
                sl = slice(64 * s, 64 * (s + 1))
                nc.scalar.activation(th[:, sl], pre[:, sl], AF.Tanh,
                                     scale=m_sb[:, t:t + 1])
            v = work.tile([128, 256], BF16, tag="v")
            nc.vector.tensor_scalar_mul(v, g_cur, a_sb[:, t:t + 1])
            g_new = state.tile([128, 256], BF16, tag="g")
            for s in range(4):
                sl = slice(64 * s, 64 * (s + 1))
                nc.vector.tensor_add(g_new[:, sl], v[:, sl], th[:, sl])
            gqt_new = state.tile([128, 256], BF16, tag="gqt")
            for s in range(4):
                sl = slice(64 * s, 64 * (s + 1))
                nc.vector.transpose(gqt_new[:, sl], g_new[:, sl])
            y_sb = work.tile([128, 256], F32, tag="y")
            nc.vector.tensor_scalar_mul(y_sb, g_new, mh_sb[:, t:t + 1])
            for j in range(4):
                nc.sync.dma_start(out=y_d[t, :, 256 * j:256 * (j + 1)],
                                  in_=y_sb[32 * j:32 * j + BS, :])
            g_cur, gqt_cur = g_new, gqt_new

        phase_a(0)
        for w in range(n_win):
            if w + 1 < n_win:
                phase_a(w + 1)
            for tl in range(min(win, t_steps - w * win)):
                scan_step(w * win + tl)

    nc.compile()
    return nc


def host_inputs(x, lengths, W_ih, W_hh, t_steps=T):
    """Common + per-core input maps."""
    x = np.asarray(x, np.float32)
    lengths = np.asarray(lengths)
    W_ih = np.asarray(W_ih, np.float32)
    W_hh = np.asarray(W_hh, np.float32)

    import ml_dtypes
    p = np.arange(128)
    wq = np.empty((128, 8 * H), np.float32)
    whh_half_T = (LEAK * W_hh).T            # [k, n] = 0.5*W_hh[n, k]
    for g in range(8):
        S = 256 * (p >> 5) + 32 * g + (p & 31)
        wq[:, g * H:(g + 1) * H] = whh_half_T[S, :]
    wq = wq.astype(ml_dtypes.bfloat16)
    wihT = np.ascontiguousarray(W_ih.T, np.float32)
    eye = np.tile(np.eye(BS, dtype=np.float32), (1, 4)).astype(ml_dtypes.bfloat16)
    id128 = np.eye(128, dtype=np.float32)

    t_idx = np.arange(t_steps)
    in_maps = []
    for c in range(NCORES):
        b0 = c * BS
        xc = np.ascontiguousarray(
            x[:t_steps, b0:b0 + BS, :].reshape(t_steps * BS, I_IN))
        lens = lengths[b0:b0 + BS].astype(np.int64)
        m_b = (t_idx[None, :] < lens[:, None]).astype(np.float32)  # [BS, T]
        m_tbl = np.empty((128, t_steps), np.float32)
        for j in range(4):
            for bp in range(4):
                r0 = 32 * j + 8 * bp
                m_tbl[r0:r0 + 8, :] = m_b
        in_maps.append({
            "x": xc, "wq": wq, "wihT": wihT,
            "m_tbl": m_tbl, "a_tbl": 1.0 - 0.5 * m_tbl, "mh_tbl": 0.5 * m_tbl,
            "eye8x32": eye, "id128": id128,
        })
    return in_maps


_CACHE = {}


def _get_program(t_steps=T, win=WIN):
    key = (t_steps, win)
    if key not in _CACHE:
        _CACHE[key] = build_program(t_steps, win)
    return _CACHE[key]


def run_on_hw(in_maps, t_steps=T, win=WIN, trace=False, tmpdir=None):
    from concourse.bass_utils import run_bass_kernel_spmd
    nc = _get_program(t_steps, win)
    return run_bass_kernel_spmd(nc, in_maps, list(range(NCORES)), trace=trace,
                                tmpdir=tmpdir)


def kernel(x, lengths, W_ih, W_hh):
    in_maps = host_inputs(x, lengths, W_ih, W_hh)
    res = run_on_hw(in_maps)
    y = np.empty((T, B, H), np.float32)
    for c in range(NCORES):
        y[:, c * BS:(c + 1) * BS, :] = res.results[c]["y"]
    return y


# revision 21
# speedup vs baseline: 1.0416x; 1.0416x over previous
"""Trainium2 Bass kernel for the ESN (leaky-tanh echo state network) scan.

Problem: y[t] = mask_t * h_t ;  h_t = (1-L)h + L*tanh(x_t@W_ih.T + h@W_hh.T), L=0.5
Shapes: x [1000, 64, 80], lengths [64] (descending), W_ih [1024, 80], W_hh [1024, 1024].

Strategy (data-parallel over batch, 8 rows/core on 8 cores):
  The scan is PE-ingest bound on W_hh (1M elements/step). W_hh.T streams as the
  MOVING operand in float32r across 4 concurrent column-tiled matmul groups
  (tile_position=(0,32j)), each producing a 256-wide h' chunk for all batch rows.
  State g = 2h is kept in a "quad" layout g_q[32j+8r+b, n] = g[b, 256j+n]
  (4 replicas r so all 128 partitions hold real data), and its 32x32
  block-transpose gQT feeds the matmuls as the stationary operand; the W rows are
  pre-permuted on the host to match the block-transposed index order.
  i2h = x@W_ih.T is folded into the PSUM accumulation via small identity matmuls;
  masking is folded into tanh's per-partition scale (tanh(m*pre)) and the leak
  into the weights (g = 2h absorbs the outer 0.5).
  i2h is computed on-device (phase A) one window ahead into DRAM-pool tiles.
"""
import sys
import os

sys.path.insert(0, '/opt/trn_rl_repo')

import numpy as np
import concourse.bass as bass
import concourse.bacc as bacc
import concourse.mybir as mybir
import concourse.tile as tile
from contextlib import ExitStack

F32 = mybir.dt.float32
F32R = mybir.dt.float32r
BF16 = mybir.dt.bfloat16
AF = mybir.ActivationFunctionType

T, B, I_IN, H = 1000, 64, 80, 1024
NCORES = 8
BS = B // NCORES      # batch rows per core
LEAK = 0.5
WIN = 64              # i2h pipeline window (time steps)
DB = 4                # DMA batching (steps per i2h load / y store)


def build_program(t_steps=T, win=WIN):
    nc = bacc.Bacc(None, target_bir_lowering=False)

    x_d = nc.dram_tensor("x", [t_steps * BS, I_IN], F32R, kind="ExternalInput")
    wq_d = nc.dram_tensor("wq", [128, 8 * H], BF16, kind="ExternalInput")
    wihT_d = nc.dram_tensor("wihT", [I_IN, H], F32R, kind="ExternalInput")
    mt_d = nc.dram_tensor("m_tbl", [128, t_steps], F32, kind="ExternalInput")
    at_d = nc.dram_tensor("a_tbl", [128, t_steps], F32, kind="ExternalInput")
    ht_d = nc.dram_tensor("mh_tbl", [128, t_steps], F32, kind="ExternalInput")
    eye_d = nc.dram_tensor("eye8x32", [BS, 32], BF16, kind="ExternalInput")
    id128_d = nc.dram_tensor("id128", [128, 128], F32R, kind="ExternalInput")
    y_d = nc.dram_tensor("y", [t_steps, BS, H], F32, kind="ExternalOutput")

    n_win = (t_steps + win - 1) // win

    with tile.TileContext(nc) as tc, ExitStack() as ctx:
        consts = ctx.enter_context(tc.tile_pool(name="consts", bufs=1))
        state = ctx.enter_context(tc.tile_pool(name="state", bufs=2))
        work = ctx.enter_context(tc.tile_pool(name="work", bufs=2))
        i2hsb = ctx.enter_context(tc.tile_pool(name="i2hsb", bufs=3))
        xload = ctx.enter_context(tc.tile_pool(name="xload", bufs=3))
        prep = ctx.enter_context(tc.tile_pool(name="prep", bufs=2, space="PSUM"))
        apsum = ctx.enter_context(tc.tile_pool(name="apsum", bufs=1, space="PSUM"))
        tpsum = ctx.enter_context(tc.tile_pool(name="tpsum", bufs=2, space="PSUM"))
        i2hdram = ctx.enter_context(tc.tile_pool(name="i2hdram", bufs=3, space="DRAM"))

        # ---- constants ----
        wq_sb = consts.tile([128, 8 * H], BF16)
        nc.sync.dma_start(out=wq_sb, in_=wq_d[:, :])
        wihT_sb = consts.tile([I_IN, H], F32R)
        nc.sync.dma_start(out=wihT_sb, in_=wihT_d[:, :])
        m_sb = consts.tile([128, t_steps], F32)
        nc.sync.dma_start(out=m_sb, in_=mt_d[:, :])
        a_sb = consts.tile([128, t_steps], F32)
        nc.sync.dma_start(out=a_sb, in_=at_d[:, :])
        mh_sb = consts.tile([128, t_steps], F32)
        nc.sync.dma_start(out=mh_sb, in_=ht_d[:, :])
        eye_sb = consts.tile([BS, 32], BF16)
        nc.sync.dma_start(out=eye_sb, in_=eye_d[:, :])
        id_sb = consts.tile([128, 128], F32R)
        nc.sync.dma_start(out=id_sb, in_=id128_d[:, :])
        zl_sb = consts.tile([1, 128], BF16)
        nc.vector.memset(zl_sb, 0.0)
        zr_sb = consts.tile([1, 256], BF16)
        nc.vector.memset(zr_sb, 0.0)

        # ---- initial state ----
        g_cur = state.tile([128, 256], BF16, tag="g")
        nc.vector.memset(g_cur, 0.0)
        gqt_cur = state.tile([128, 256], BF16, tag="gqt")
        nc.vector.memset(gqt_cur, 0.0)
        # one-time DVE reads of the step tables so their DMA waits don't land
        # on per-step TensorScalarPtr ops (which allow a single sync wait).
        tbl_warm = consts.tile([128, 4], F32)
        nc.vector.tensor_copy(tbl_warm[:, 0:1], m_sb[:, 0:1])
        nc.vector.tensor_copy(tbl_warm[:, 1:2], a_sb[:, 0:1])
        nc.vector.tensor_copy(tbl_warm[:, 2:3], mh_sb[:, 0:1])

        win_tiles = {}
        i2h_cur = None
        y_blk = None

        def phase_a(w):
            t0 = w * win
            nt = min(win, t_steps - t0)
            rows = nt * BS
            dtile = i2hdram.tile([win * BS, H], BF16, tag="i2hwin")
            win_tiles[w] = dtile
            r = 0
            while r < rows:
                cr = min(128, rows - r)
                x_sb = xload.tile([128, I_IN], F32R, tag="xc")
                nc.sync.dma_start(out=x_sb[:cr, :],
                                  in_=x_d[t0 * BS + r: t0 * BS + r + cr, :])
                xt_ps = tpsum.tile([I_IN, 128], F32R, tag="xtp")
                nc.tensor.transpose(xt_ps[:, :cr], x_sb[:cr, :I_IN],
                                    id_sb[:cr, :cr])
                xt_sb = xload.tile([I_IN, 128], F32R, tag="xt")
                nc.scalar.copy(out=xt_sb[:, :cr], in_=xt_ps[:, :cr])
                for half in range(2):
                    psh = apsum.tile([128, 512], F32, tag=f"aps{half}")
                    nc.tensor.matmul(
                        psh[:cr, :],
                        xt_sb[:, :cr],
                        wihT_sb[:, half * 512:(half + 1) * 512],
                        start=True, stop=True,
                    )
                    cp = xload.tile([128, 512], BF16, tag=f"i2hcp{half}")
                    if half == 0:
                        nc.scalar.copy(out=cp[:cr, :], in_=psh[:cr, :])
                    else:
                        nc.vector.tensor_copy(cp[:cr, :], psh[:cr, :])
                    nc.sync.dma_start(
                        out=dtile[r:r + cr, half * 512:(half + 1) * 512],
                        in_=cp[:cr, :])
                r += cr

        def scan_step(t):
            nonlocal g_cur, gqt_cur, i2h_cur, y_blk
            w, tl = divmod(t, win)
            kb = t % DB
            if kb == 0:
                dtile = win_tiles[w]
                i2h_cur = i2hsb.tile([BS, DB * H], BF16, tag="i2h")
                nc.sync.dma_start(
                    out=i2h_cur.rearrange("b (k n) -> b k n", k=DB),
                    in_=dtile[tl * BS:(tl + DB) * BS, :].rearrange(
                        "(k b) n -> b k n", b=BS))
                y_blk = work.tile([128, DB * 256], F32, tag="y")

            pre = prep.tile([128, 256], F32, tag="pre")
            # full-width bf16 zero-matmul claims the psum bank: one start=True
            # covering all 128 partitions; the four interleaved col-group
            # chains use start=False (per-element has_written overwrite).
            nc.tensor.matmul(pre[:, :], zl_sb[:, :], zr_sb[0:1, 0:256],
                             start=True, stop=False, tile_position=(0, 0))
            for g in range(8):
                for j in range(4):
                    nc.tensor.matmul(
                        pre[32 * j:32 * j + 32, :],
                        gqt_cur[:, 32 * g:32 * g + 32],
                        wq_sb[:, g * H + j * 256:g * H + j * 256 + 256],
                        start=False, stop=False,
                        tile_position=(0, 32 * j),
                    )
            for j in range(4):
                nc.tensor.matmul(
                    pre[32 * j:32 * j + 32, :],
                    eye_sb[:, :],
                    i2h_cur[:, kb * H + j * 256:kb * H + (j + 1) * 256],
                    start=False, stop=False,
                    tile_position=(0, 32 * j),
                )
            nc.tensor.matmul(pre[:, 0:1], zl_sb[:, :], zr_sb[0:1, 0:1],
                             start=False, stop=True, tile_position=(0, 0))

            th = work.tile([128, 256], F32, tag="th")
            for s in range(4):
                sl = slice(64 * s, 64 * (s + 1))
                nc.scalar.activation(th[:, sl], pre[:, sl], AF.Tanh,
                                     scale=m_sb[:, t:t + 1])
            v = work.tile([128, 256], BF16, tag="v")
            nc.vector.tensor_scalar_mul(v, g_cur, a_sb[:, t:t + 1])
            g_new = state.tile([128, 256], BF16, tag="g")
            for s in range(4):
                sl = slice(64 * s, 64 * (s + 1))
                nc.vector.tensor_add(g_new[:, sl], v[:, sl], th[:, sl])
            gqt_new = state.tile([128, 256], BF16, tag="gqt")
            for s in range(4):
                sl = slice(64 * s, 64 * (s + 1))
                nc.vector.transpose(gqt_new[:, sl], g_new[:, sl])
            nc.vector.tensor_scalar_mul(y_blk[:, kb * 256:(kb + 1) * 256],
                                        g_new, mh_sb[:, t:t + 1])
            if kb == DB - 1:
                t0b = t - DB + 1
                for j in range(4):
                    nc.sync.dma_start(
                        out=y_d[t0b:t0b + DB, :,
                                256 * j:256 * (j + 1)].transpose([1, 0, 2]),
                        in_=y_blk[32 * j:32 * j + BS, :].rearrange(
                            "b (k n) -> b k n", k=DB))
            g_cur, gqt_cur = g_new, gqt_new

        phase_a(0)
        for w in range(n_win):
            if w + 1 < n_win:
                phase_a(w + 1)
            for tl in range(min(win, t_steps - w * win)):
                scan_step(w * win + tl)

    nc.compile()
    return nc


def host_inputs(x, lengths, W_ih, W_hh, t_steps=T):
    """Common + per-core input maps."""
    x = np.asarray(x, np.float32)
    lengths = np.asarray(lengths)
    W_ih = np.asarray(W_ih, np.float32)
    W_hh = np.asarray(W_hh, np.float32)

    import ml_dtypes
    p = np.arange(128)
    wq = np.empty((128, 8 * H), np.float32)
    whh_half_T = (LEAK * W_hh).T            # [k, n] = 0.5*W_hh[n, k]
    for g in range(8):
        S = 256 * (p >> 5) + 32 * g + (p & 31)
        wq[:, g * H:(g + 1) * H] = whh_half_T[S, :]
    wq = wq.astype(ml_dtypes.bfloat16)
    wihT = np.ascontiguousarray(W_ih.T, np.float32)
    eye = np.tile(np.eye(BS, dtype=np.float32), (1, 4)).astype(ml_dtypes.bfloat16)
    id128 = np.eye(128, dtype=np.float32)

    t_idx = np.arange(t_steps)
    in_maps = []
    for c in range(NCORES):
        b0 = c * BS
        xc = np.ascontiguousarray(
            x[:t_steps, b0:b0 + BS, :].reshape(t_steps * BS, I_IN))
        lens = lengths[b0:b0 + BS].astype(np.int64)
        m_b = (t_idx[None, :] < lens[:, None]).astype(np.float32)  # [BS, T]
        m_tbl = np.empty((128, t_steps), np.float32)
        for j in range(4):
            for bp in range(4):
                r0 = 32 * j + 8 * bp
                m_tbl[r0:r0 + 8, :] = m_b
        in_maps.append({
            "x": xc, "wq": wq, "wihT": wihT,
            "m_tbl": m_tbl, "a_tbl": 1.0 - 0.5 * m_tbl, "mh_tbl": 0.5 * m_tbl,
            "eye8x32": eye, "id128": id128,
        })
    return in_maps


_CACHE = {}


def _get_program(t_steps=T, win=WIN):
    key = (t_steps, win)
    if key not in _CACHE:
        _CACHE[key] = build_program(t_steps, win)
    return _CACHE[key]


def run_on_hw(in_maps, t_steps=T, win=WIN, trace=False, tmpdir=None):
    from concourse.bass_utils import run_bass_kernel_spmd
    nc = _get_program(t_steps, win)
    return run_bass_kernel_spmd(nc, in_maps, list(range(NCORES)), trace=trace,
                                tmpdir=tmpdir)


def kernel(x, lengths, W_ih, W_hh):
    in_maps = host_inputs(x, lengths, W_ih, W_hh)
    res = run_on_hw(in_maps)
    y = np.empty((T, B, H), np.float32)
    for c in range(NCORES):
        y[:, c * BS:(c + 1) * BS, :] = res.results[c]["y"]
    return y
